# revision 2
# baseline (speedup 1.0000x reference)
"""AttnBlock (GroupNorm -> QKV -> full 1024-token spatial attention -> out-proj
-> residual) for B=32, H=W=32, C=512 on 8 Trainium2 NeuronCores.

Sharding: data-parallel over batch (4 batch elements per core). Everything on
one core is a single Bass/Tile program:

  per batch element b (activations as [tokens=1024, C=512]):
    x      -> PE-transpose -> xT [C-part, tok]  (rounded to f32r)
    stats: bn_stats per channel + tiny G-matmul for per-group mean/E[x^2]
    hT = xT * A + B in place (A,B per-channel from group stats; f32r)
    QT = Wq^T hT (+bq), KT = Wk^T hT (+bk)    [C-part, tok]  f32r
    V  = hT^T Wv                              [tok-part, C]  f32r
    per 512-token chunk i:
      ET[j,i] = exp(scale * KT^T QT)          [tok_j-part, i] f32r
      l[i] = ones^T ET;  rl = 1/l transposed to partitions via K=1 matmuls
      UT = V^T ET (unnormalized PV)           [C-part, i]
      out = (UT^T Wo) * rl + bo2 + x          [tok-part, C]

All big matmuls run in float32r (TF32-like, full PE rate, ~1e-4 rel rounding),
K=128 per accumulation step. Tiny matmuls (group reduce/expand, l-transpose)
run in plain fp32. bv/bo are folded into bo2 = bv @ Wo + bo on host (softmax
rows sum to 1, so +bv on V becomes +bv on PV).
"""

import math

import numpy as np

B_TOTAL = 32
N_CORES = 8
B_PER = B_TOTAL // N_CORES
N = 1024
C = 512
G = 32
CT = 4     # channel tiles of 128
IT = 8     # token tiles of 128
ICH = 2    # token chunks of 512
EPS = 1e-6
SCALE = 1.0 / math.sqrt(C)

_CACHE = {}


def _build(use_bq, use_bk, use_bo2):
    import concourse.tile as tile
    from concourse import bacc, mybir
    from concourse.masks import make_identity

    f32 = mybir.dt.float32
    f32r = mybir.dt.float32r
    AF = mybir.ActivationFunctionType
    ALU = mybir.AluOpType

    nc = bacc.Bacc("TRN2", target_bir_lowering=False, debug=False,
                   num_devices=N_CORES)

    xs_d = nc.dram_tensor("xs", [B_PER, N, C], f32, kind="ExternalInput").ap()
    w_d = {
        name: nc.dram_tensor(name, [C, C], f32r, kind="ExternalInput").ap()
        for name in ("wq", "wk", "wv", "wo")
    }
    g4_d = nc.dram_tensor("g4", [128, CT * G], f32, kind="ExternalInput").ap()
    e4_d = nc.dram_tensor("e4", [G, CT * 128], f32, kind="ExternalInput").ap()
    ones_d = nc.dram_tensor("ones_in", [128, 1], f32r, kind="ExternalInput").ap()
    gns_d = nc.dram_tensor("gnsc", [128, CT], f32, kind="ExternalInput").ap()
    gnb_d = nc.dram_tensor("gnbc", [128, CT], f32, kind="ExternalInput").ap()
    bq_d = nc.dram_tensor("bqc", [128, CT], f32, kind="ExternalInput").ap() if use_bq else None
    bk_d = nc.dram_tensor("bkc", [128, CT], f32, kind="ExternalInput").ap() if use_bk else None
    bo2_d = nc.dram_tensor("bo2bc", [128, C], f32, kind="ExternalInput").ap() if use_bo2 else None
    out_d = nc.dram_tensor("out", [B_PER, N, C], f32, kind="ExternalOutput").ap()

    with tile.TileContext(nc) as tc:
        with (
            tc.tile_pool(name="consts", bufs=1) as consts,
            tc.tile_pool(name="xp", bufs=2) as xp,
            tc.tile_pool(name="htp", bufs=2) as htp,
            tc.tile_pool(name="qtp", bufs=1) as qtp,
            tc.tile_pool(name="ktp", bufs=1) as ktp,
            tc.tile_pool(name="vp", bufs=1) as vp,
            tc.tile_pool(name="ep", bufs=1) as ep,
            tc.tile_pool(name="utp", bufs=2) as utp,
            tc.tile_pool(name="op", bufs=3) as op,
            tc.tile_pool(name="statp", bufs=2) as statp,
            tc.tile_pool(name="pp", bufs=4, space="PSUM") as pp,
            tc.tile_pool(name="sp", bufs=2, space="PSUM") as sp,
        ):
            # ---- per-core constants
            wq = [consts.tile([128, C], f32r, name=f"wq{i}", tag=f"wq{i}") for i in range(CT)]
            wk = [consts.tile([128, C], f32r, name=f"wk{i}", tag=f"wk{i}") for i in range(CT)]
            wv = [consts.tile([128, C], f32r, name=f"wv{i}", tag=f"wv{i}") for i in range(CT)]
            wo = [consts.tile([128, C], f32r, name=f"wo{i}", tag=f"wo{i}") for i in range(CT)]
            for i in range(CT):
                nc.sync.dma_start(wq[i][:], w_d["wq"][i * 128:(i + 1) * 128, :])
                nc.sync.dma_start(wk[i][:], w_d["wk"][i * 128:(i + 1) * 128, :])
                nc.sync.dma_start(wv[i][:], w_d["wv"][i * 128:(i + 1) * 128, :])
                nc.sync.dma_start(wo[i][:], w_d["wo"][i * 128:(i + 1) * 128, :])
            g4 = consts.tile([128, CT * G], f32)
            nc.sync.dma_start(g4[:], g4_d[:])
            e4 = consts.tile([G, CT * 128], f32)
            nc.sync.dma_start(e4[:], e4_d[:])
            ones_r = consts.tile([128, 1], f32r)
            nc.sync.dma_start(ones_r[:], ones_d[:])
            gnsc = consts.tile([128, CT], f32)
            nc.sync.dma_start(gnsc[:], gns_d[:])
            gnbc = consts.tile([128, CT], f32)
            nc.sync.dma_start(gnbc[:], gnb_d[:])
            if use_bq:
                bqc = consts.tile([128, CT], f32)
                nc.sync.dma_start(bqc[:], bq_d[:])
            if use_bk:
                bkc = consts.tile([128, CT], f32)
                nc.sync.dma_start(bkc[:], bk_d[:])
            if use_bo2:
                bo2bc = consts.tile([128, C], f32)
                nc.sync.dma_start(bo2bc[:], bo2_d[:])
            ident = consts.tile([128, 128], f32)
            make_identity(nc, ident[:])
            onef = consts.tile([128, 1], f32)
            nc.vector.memset(onef[:], 1.0)
            eps32 = consts.tile([G, 1], f32)
            nc.vector.memset(eps32[:], EPS)

            for b in range(B_PER):
                # ---- load x (natural layout, tokens on partitions)
                x_sb = xp.tile([128, IT, C], f32, tag="x")
                nc.sync.dma_start(
                    x_sb[:], xs_d[b].rearrange("(t p) c -> p t c", p=128))

                # ---- transpose x -> xT (channels on partitions), f32r-rounded
                ht = htp.tile([128, CT, N], f32r, tag="ht")
                for ct in range(CT):
                    for ig in range(2):
                        ptr = pp.tile([128, 512], f32, tag="mm")
                        for k in range(4):
                            it = ig * 4 + k
                            nc.tensor.transpose(
                                ptr[:, k * 128:(k + 1) * 128],
                                x_sb[:, it, ct * 128:(ct + 1) * 128],
                                ident[:])
                        nc.scalar.copy(ht[:, ct, ig * 512:(ig + 1) * 512], ptr[:])

                # ---- per-channel stats over the 1024 tokens
                stats = statp.tile([128, CT, 2, 6], f32, tag="stats")
                mvt = statp.tile([128, CT, 2], f32, tag="mvt")
                ms = statp.tile([128, CT, 2], f32, tag="ms")
                for ct in range(CT):
                    for h in range(2):
                        nc.vector.bn_stats(
                            stats[:, ct, h, :],
                            ht[:, ct, h * 512:(h + 1) * 512].bitcast(f32))
                    nc.vector.bn_aggr(mvt[:, ct, :], stats[:, ct, :, :])
                    nc.vector.tensor_copy(ms[:, ct, 0:1], mvt[:, ct, 0:1])
                    t1 = statp.tile([128, 1], f32, tag="t1")
                    nc.vector.tensor_mul(t1[:], mvt[:, ct, 0:1], mvt[:, ct, 0:1])
                    nc.vector.tensor_add(ms[:, ct, 1:2], mvt[:, ct, 1:2], t1[:])

                # ---- group reduce: [32, (mean, E[x^2])] = G4^T @ ms / 16
                pg = sp.tile([G, 2], f32, tag="small")
                for ct in range(CT):
                    nc.tensor.matmul(pg[:], g4[:, ct * G:(ct + 1) * G],
                                     ms[:, ct, :],
                                     start=(ct == 0), stop=(ct == CT - 1))
                gmv = statp.tile([G, 2], f32, tag="gmv")
                nc.vector.tensor_scalar_mul(gmv[:], pg[:], 1.0 / 16.0)
                m2 = statp.tile([G, 1], f32, tag="m2")
                nc.vector.tensor_mul(m2[:], gmv[:, 0:1], gmv[:, 0:1])
                var32 = statp.tile([G, 1], f32, tag="var32")
                nc.vector.tensor_tensor(
                    out=var32[:], in0=gmv[:, 1:2], in1=m2[:], op=ALU.subtract)
                std32 = statp.tile([G, 1], f32, tag="std32")
                nc.scalar.activation(std32[:], var32[:], AF.Sqrt,
                                     bias=eps32[:], scale=1.0)
                rstd32 = statp.tile([G, 1], f32, tag="rstd32")
                nc.vector.reciprocal(rstd32[:], std32[:])

                # ---- expand group stats to channels; A/B affine coefs
                acols = statp.tile([128, CT], f32, tag="acols")
                bcols = statp.tile([128, CT], f32, tag="bcols")
                for ct in range(CT):
                    pe_a = sp.tile([128, 1], f32, tag="small")
                    nc.tensor.matmul(pe_a[:], e4[:, ct * 128:(ct + 1) * 128],
                                     rstd32[:], start=True, stop=True)
                    pe_b = sp.tile([128, 1], f32, tag="small")
                    nc.tensor.matmul(pe_b[:], e4[:, ct * 128:(ct + 1) * 128],
                                     gmv[:, 0:1], start=True, stop=True)
                    nc.vector.tensor_mul(acols[:, ct:ct + 1], gnsc[:, ct:ct + 1],
                                         pe_a[:])
                    t2 = statp.tile([128, 1], f32, tag="t2")
                    nc.vector.tensor_mul(t2[:], acols[:, ct:ct + 1], pe_b[:])
                    nc.vector.tensor_tensor(
                        out=bcols[:, ct:ct + 1], in0=gnbc[:, ct:ct + 1],
                        in1=t2[:], op=ALU.subtract)

                # ---- hT = xT * A + B (in place, per channel tile)
                for ct in range(CT):
                    nc.vector.tensor_scalar(
                        ht[:, ct, :], ht[:, ct, :].bitcast(f32),
                        acols[:, ct:ct + 1], bcols[:, ct:ct + 1],
                        op0=ALU.mult, op1=ALU.add)

                # ---- projections
                qt = qtp.tile([128, CT, N], f32r, tag="qt")
                kt = ktp.tile([128, CT, N], f32r, tag="kt")
                for dst, w, bias in ((qt, wq, bqc if use_bq else None),
                                     (kt, wk, bkc if use_bk else None)):
                    for ct in range(CT):
                        for ich in range(ICH):
                            pq = pp.tile([128, 512], f32, tag="mm")
                            for cp in range(CT):
                                nc.tensor.matmul(
                                    pq[:],
                                    w[cp][:, ct * 128:(ct + 1) * 128],
                                    ht[:, cp, ich * 512:(ich + 1) * 512],
                                    start=(cp == 0), stop=(cp == CT - 1))
                            dslice = dst[:, ct, ich * 512:(ich + 1) * 512]
                            if bias is not None:
                                nc.scalar.activation(
                                    dslice, pq[:], AF.Identity,
                                    bias=bias[:, ct:ct + 1], scale=1.0)
                            else:
                                nc.scalar.copy(dslice, pq[:])
                v = vp.tile([128, IT, C], f32r, tag="v")
                for it in range(IT):
                    pv = pp.tile([128, 512], f32, tag="mm")
                    for cp in range(CT):
                        nc.tensor.matmul(
                            pv[:], ht[:, cp, it * 128:(it + 1) * 128],
                            wv[cp][:], start=(cp == 0), stop=(cp == CT - 1))
                    nc.vector.tensor_copy(v[:, it, :], pv[:])

                # ---- attention, one 512-token chunk of queries at a time
                for ich in range(ICH):
                    e_t = ep.tile([128, IT, 512], f32r, tag="et")
                    for jt in range(IT):
                        s_ps = pp.tile([128, 512], f32, tag="mm")
                        for cp in range(CT):
                            nc.tensor.matmul(
                                s_ps[:],
                                kt[:, cp, jt * 128:(jt + 1) * 128],
                                qt[:, cp, ich * 512:(ich + 1) * 512],
                                start=(cp == 0), stop=(cp == CT - 1))
                        nc.scalar.activation(e_t[:, jt, :], s_ps[:], AF.Exp,
                                             bias=0.0, scale=SCALE)

                    pl = sp.tile([1, 512], f32, tag="small")
                    for jt in range(IT):
                        nc.tensor.matmul(pl[:], ones_r[:], e_t[:, jt, :],
                                         start=(jt == 0), stop=(jt == IT - 1))
                    lsb = statp.tile([1, 512], f32, tag="lsb")
                    nc.scalar.copy(lsb[:], pl[:])
                    rl = statp.tile([128, 4], f32, tag="rl")
                    for k in range(4):
                        plt = sp.tile([128, 1], f32, tag="small")
                        nc.tensor.matmul(plt[:], lsb[0:1, k * 128:(k + 1) * 128],
                                         onef[0:1, 0:1], start=True, stop=True)
                        nc.vector.reciprocal(rl[:, k:k + 1], plt[:])

                    ut = utp.tile([128, CT, 512], f32r, tag="ut")
                    for ct in range(CT):
                        pu = pp.tile([128, 512], f32, tag="mm")
                        for jt in range(IT):
                            nc.tensor.matmul(
                                pu[:], v[:, jt, ct * 128:(ct + 1) * 128],
                                e_t[:, jt, :],
                                start=(jt == 0), stop=(jt == IT - 1))
                        nc.vector.tensor_copy(ut[:, ct, :], pu[:])

                    for k in range(4):
                        it = ich * 4 + k
                        po = pp.tile([128, 512], f32, tag="mm")
                        for ct in range(CT):
                            nc.tensor.matmul(
                                po[:], ut[:, ct, k * 128:(k + 1) * 128],
                                wo[ct][:], start=(ct == 0), stop=(ct == CT - 1))
                        o_sb = op.tile([128, C], f32, tag="osb")
                        nc.scalar.activation(o_sb[:], po[:], AF.Copy,
                                             bias=0.0, scale=rl[:, k:k + 1])
                        o2 = op.tile([128, C], f32, tag="o2")
                        if use_bo2:
                            nc.vector.tensor_add(o_sb[:], o_sb[:], bo2bc[:])
                        nc.vector.tensor_add(o2[:], o_sb[:], x_sb[:, it, :])
                        nc.sync.dma_start(
                            out_d[b, it * 128:(it + 1) * 128, :], o2[:])

    nc.compile()
    return nc


def _host_consts():
    g4 = np.zeros((128, CT * G), np.float32)
    e4 = np.zeros((G, CT * 128), np.float32)
    for ct in range(CT):
        for p in range(128):
            g = ct * 8 + p // 16
            g4[p, ct * G + g] = 1.0
            e4[g, ct * 128 + p] = 1.0
    return g4, e4, np.ones((128, 1), np.float32)


def kernel(**inputs):
    from concourse import bass_utils

    x = np.ascontiguousarray(np.asarray(inputs["x"], np.float32))
    gn_scale = np.asarray(inputs["gn_scale"], np.float32)
    gn_bias = np.asarray(inputs["gn_bias"], np.float32)
    Wq = np.ascontiguousarray(np.asarray(inputs["Wq"], np.float32))
    Wk = np.ascontiguousarray(np.asarray(inputs["Wk"], np.float32))
    Wv = np.ascontiguousarray(np.asarray(inputs["Wv"], np.float32))
    Wo = np.ascontiguousarray(np.asarray(inputs["Wo"], np.float32))
    bq = np.asarray(inputs["bq"], np.float32)
    bk = np.asarray(inputs["bk"], np.float32)
    bv = np.asarray(inputs["bv"], np.float32)
    bo = np.asarray(inputs["bo"], np.float32)

    B, H, W, Cc = x.shape
    assert (B, H * W, Cc) == (B_TOTAL, N, C)

    bo2 = bv @ Wo + bo
    use_bq = bool(np.any(bq))
    use_bk = bool(np.any(bk))
    use_bo2 = bool(np.any(bo2))

    key = (use_bq, use_bk, use_bo2)
    if key not in _CACHE:
        _CACHE[key] = _build(*key)
    nc = _CACHE[key]

    g4, e4, ones = _host_consts()

    def cols(vec):
        return np.ascontiguousarray(vec.reshape(CT, 128).T)

    base = {
        "wq": Wq, "wk": Wk, "wv": Wv, "wo": Wo,
        "g4": g4, "e4": e4, "ones_in": ones,
        "gnsc": cols(gn_scale), "gnbc": cols(gn_bias),
    }
    if use_bq:
        base["bqc"] = cols(bq)
    if use_bk:
        base["bkc"] = cols(bk)
    if use_bo2:
        base["bo2bc"] = np.ascontiguousarray(
            np.broadcast_to(bo2[None, :], (128, C)))

    x_flat = x.reshape(B_TOTAL, N, C)
    in_maps = []
    for c in range(N_CORES):
        m = dict(base)
        m["xs"] = np.ascontiguousarray(x_flat[c * B_PER:(c + 1) * B_PER])
        in_maps.append(m)

    res = bass_utils.run_bass_kernel_spmd(nc, in_maps,
                                          core_ids=list(range(N_CORES)))
    out = np.concatenate([r["out"] for r in res.results], axis=0)
    return out.reshape(B_TOTAL, H, W, C).astype(np.float32)


# revision 17
# speedup vs baseline: 1.1252x; 1.1252x over previous
"""AttnBlock (GroupNorm -> QKV -> full 1024-token spatial attention -> out-proj
-> residual) for B=32, H=W=32, C=512 on 8 Trainium2 NeuronCores.

Sharding: data-parallel over batch (4 batch elements per core). Everything on
one core is a single Bass/Tile program:

  per batch element b (activations as [tokens=1024, C=512]):
    x      -> PE-transpose -> xT [C-part, tok]  (rounded to f32r)
    stats: bn_stats per channel + tiny G-matmul for per-group mean/E[x^2]
    hT = xT * A + B in place (A,B per-channel from group stats; f32r)
    QT = Wq^T hT (+bq), KT = Wk^T hT (+bk)    [C-part, tok]  f32r
    V  = hT^T Wv                              [tok-part, C]  f32r
    per 512-token chunk i:
      ET[j,i] = exp(scale * KT^T QT)          [tok_j-part, i] f32r
      l[i] = ones^T ET;  rl = 1/l transposed to partitions via K=1 matmuls
      UT = V^T ET (unnormalized PV)           [C-part, i]
      out = (UT^T Wo) * rl + bo2 + x          [tok-part, C]

All big matmuls run in float32r (TF32-like, full PE rate, ~1e-4 rel rounding),
K=128 per accumulation step. Tiny matmuls (group reduce/expand, l-transpose)
run in plain fp32. bv/bo are folded into bo2 = bv @ Wo + bo on host (softmax
rows sum to 1, so +bv on V becomes +bv on PV).
"""

import math

import numpy as np

B_TOTAL = 32
N_CORES = 8
B_PER = B_TOTAL // N_CORES
N = 1024
C = 512
G = 32
CT = 4     # channel tiles of 128
IT = 8     # token tiles of 128
ICH = 2    # token chunks of 512
EPS = 1e-6
SCALE = 1.0 / math.sqrt(C)

_CACHE = {}


def _build(use_bq, use_bk, use_bo2):
    import concourse.tile as tile
    from concourse import bacc, mybir
    f32 = mybir.dt.float32
    f32r = mybir.dt.float32r
    AF = mybir.ActivationFunctionType
    ALU = mybir.AluOpType

    nc = bacc.Bacc("TRN2", target_bir_lowering=False, debug=False,
                   num_devices=N_CORES)

    xs_d = nc.dram_tensor("xs", [B_PER, N, C], f32r, kind="ExternalInput").ap()
    w_d = {
        name: nc.dram_tensor(name, [C, C], f32r, kind="ExternalInput").ap()
        for name in ("wq", "wk", "wv", "wo")
    }
    g4_d = nc.dram_tensor("g4", [128, CT * G], f32, kind="ExternalInput").ap()
    e4_d = nc.dram_tensor("e4", [G, CT * 128], f32, kind="ExternalInput").ap()
    ones_d = nc.dram_tensor("ones_in", [128, 1], f32r, kind="ExternalInput").ap()
    id_d = nc.dram_tensor("ident_in", [128, 128], f32r, kind="ExternalInput").ap()
    gns_d = nc.dram_tensor("gnsc", [128, CT], f32, kind="ExternalInput").ap()
    gnb_d = nc.dram_tensor("gnbc", [128, CT], f32, kind="ExternalInput").ap()
    bq_d = nc.dram_tensor("bqc", [128, CT], f32, kind="ExternalInput").ap() if use_bq else None
    bk_d = nc.dram_tensor("bkc", [128, CT], f32, kind="ExternalInput").ap() if use_bk else None
    bo2_d = nc.dram_tensor("bo2bc", [128, C], f32, kind="ExternalInput").ap() if use_bo2 else None
    out_d = nc.dram_tensor("out", [B_PER, N, C], f32, kind="ExternalOutput").ap()

    with tile.TileContext(nc) as tc:
        with (
            tc.tile_pool(name="consts", bufs=1) as consts,
            tc.tile_pool(name="xp", bufs=2) as xp,
            tc.tile_pool(name="htp", bufs=2) as htp,
            tc.tile_pool(name="qtp", bufs=1) as qtp,
            tc.tile_pool(name="ktp", bufs=1) as ktp,
            tc.tile_pool(name="vp", bufs=1) as vp,
            tc.tile_pool(name="ep", bufs=1) as ep,
            tc.tile_pool(name="utp", bufs=2) as utp,
            tc.tile_pool(name="op", bufs=3) as op,
            tc.tile_pool(name="statp", bufs=2) as statp,
            tc.tile_pool(name="pp", bufs=6, space="PSUM") as pp,
            tc.tile_pool(name="sp", bufs=2, space="PSUM") as sp,
        ):
            # ---- first batch's x load goes ahead of the weight DMAs so the
            # PE transposes can start while weights stream in
            ident = consts.tile([128, 128], f32r)
            nc.sync.dma_start(ident[:], id_d[:])
            x_tiles = {}
            x_tiles[0] = xp.tile([128, IT, C], f32r, name="x0", tag="x")
            for it in range(IT):
                nc.sync.dma_start(x_tiles[0][:, it, :],
                                  xs_d[0, it * 128:(it + 1) * 128, :])
            # warm up the PE clock (HAM) while the first x tiles stream in
            wu = pp.tile([128, 512], f32r, name="wu", tag="mm")
            for i in range(10):
                nc.tensor.transpose(wu[:, (i % 4) * 128:(i % 4 + 1) * 128],
                                    ident[:], ident[:])

            # ---- small consts first (needed by the batch-0 stats chain),
            # then weights in consumption order wq -> wk -> wv -> wo
            g4 = consts.tile([128, CT * G], f32)
            nc.gpsimd.dma_start(g4[:], g4_d[:])
            e4 = consts.tile([G, CT * 128], f32)
            nc.gpsimd.dma_start(e4[:], e4_d[:])
            ones_r = consts.tile([128, 1], f32r)
            nc.gpsimd.dma_start(ones_r[:], ones_d[:])
            gnsc = consts.tile([128, CT], f32)
            nc.gpsimd.dma_start(gnsc[:], gns_d[:])
            gnbc = consts.tile([128, CT], f32)
            nc.gpsimd.dma_start(gnbc[:], gnb_d[:])
            if use_bq:
                bqc = consts.tile([128, CT], f32)
                nc.gpsimd.dma_start(bqc[:], bq_d[:])
            if use_bk:
                bkc = consts.tile([128, CT], f32)
                nc.gpsimd.dma_start(bkc[:], bk_d[:])
            if use_bo2:
                bo2bc = consts.tile([128, C], f32)
                nc.gpsimd.dma_start(bo2bc[:], bo2_d[:])
            onef = consts.tile([128, 1], f32)
            nc.vector.memset(onef[:], 1.0)
            eps32 = consts.tile([G, 1], f32)
            nc.vector.memset(eps32[:], EPS)
            wq = [consts.tile([128, C], f32r, name=f"wq{i}", tag=f"wq{i}") for i in range(CT)]
            wk = [consts.tile([128, C], f32r, name=f"wk{i}", tag=f"wk{i}") for i in range(CT)]
            wv = [consts.tile([128, C], f32r, name=f"wv{i}", tag=f"wv{i}") for i in range(CT)]
            wo = [consts.tile([128, C], f32r, name=f"wo{i}", tag=f"wo{i}") for i in range(CT)]
            for w, nm in ((wq, "wq"), (wk, "wk"), (wv, "wv"), (wo, "wo")):
                for i in range(CT):
                    nc.gpsimd.dma_start(w[i][:], w_d[nm][i * 128:(i + 1) * 128, :])

            ht_tiles = {}

            def load_x(b):
                if b not in x_tiles:
                    x_sb = xp.tile([128, IT, C], f32r, name="x_sb", tag="x")
                    for it in range(IT):
                        nc.sync.dma_start(x_sb[:, it, :],
                                          xs_d[b, it * 128:(it + 1) * 128, :])
                    x_tiles[b] = x_sb
                return x_tiles[b]

            def phase1a(b):
                # transpose x -> ht (f32r-rounded), token-group major so the
                # first transposes only need the first half of x
                x_sb = load_x(b)
                ht = htp.tile([128, CT, N], f32r, name="ht", tag="ht")
                ht_tiles[b] = ht
                for ig in range(2):
                    for ct in range(CT):
                        ptr = pp.tile([128, 512], f32r, name="ptr", tag="mm")
                        for k in range(4):
                            it = ig * 4 + k
                            nc.tensor.transpose(
                                ptr[:, k * 128:(k + 1) * 128],
                                x_sb[:, it, ct * 128:(ct + 1) * 128],
                                ident[:])
                        nc.scalar.copy(ht[:, ct, ig * 512:(ig + 1) * 512], ptr[:])

            def phase1b(b):
                # groupnorm stats + in-place affine on ht
                ht = ht_tiles[b]
                # per-channel stats over the 1024 tokens
                stats = statp.tile([128, CT, 2, 6], f32, name="stats", tag="stats")
                mvt = statp.tile([128, CT, 2], f32, name="mvt", tag="mvt")
                ms = statp.tile([128, CT, 2], f32, name="ms", tag="ms")
                for ct in range(CT):
                    for h in range(2):
                        nc.vector.bn_stats(
                            stats[:, ct, h, :],
                            ht[:, ct, h * 512:(h + 1) * 512].bitcast(f32))
                    nc.vector.bn_aggr(mvt[:, ct, :], stats[:, ct, :, :])
                    nc.vector.tensor_copy(ms[:, ct, 0:1], mvt[:, ct, 0:1])
                    t1 = statp.tile([128, 1], f32, tag="t1")
                    nc.vector.tensor_mul(t1[:], mvt[:, ct, 0:1], mvt[:, ct, 0:1])
                    nc.vector.tensor_add(ms[:, ct, 1:2], mvt[:, ct, 1:2], t1[:])

                # ---- group reduce: [32, (mean, E[x^2])] = G4^T @ ms / 16
                pg = sp.tile([G, 2], f32, tag="small")
                for ct in range(CT):
                    nc.tensor.matmul(pg[:], g4[:, ct * G:(ct + 1) * G],
                                     ms[:, ct, :],
                                     start=(ct == 0), stop=(ct == CT - 1))
                gmv = statp.tile([G, 2], f32, tag="gmv")
                nc.vector.tensor_scalar_mul(gmv[:], pg[:], 1.0 / 16.0)
                m2 = statp.tile([G, 1], f32, tag="m2")
                nc.vector.tensor_mul(m2[:], gmv[:, 0:1], gmv[:, 0:1])
                var32 = statp.tile([G, 1], f32, tag="var32")
                nc.vector.tensor_tensor(
                    out=var32[:], in0=gmv[:, 1:2], in1=m2[:], op=ALU.subtract)
                std32 = statp.tile([G, 1], f32, tag="std32")
                nc.scalar.activation(std32[:], var32[:], AF.Sqrt,
                                     bias=eps32[:], scale=1.0)
                rstd32 = statp.tile([G, 1], f32, tag="rstd32")
                nc.vector.reciprocal(rstd32[:], std32[:])

                # ---- expand group stats to channels; A/B affine coefs
                acols = statp.tile([128, CT], f32, tag="acols")
                bcols = statp.tile([128, CT], f32, tag="bcols")
                for ct in range(CT):
                    pe_a = sp.tile([128, 1], f32, tag="small")
                    nc.tensor.matmul(pe_a[:], e4[:, ct * 128:(ct + 1) * 128],
                                     rstd32[:], start=True, stop=True)
                    pe_b = sp.tile([128, 1], f32, tag="small")
                    nc.tensor.matmul(pe_b[:], e4[:, ct * 128:(ct + 1) * 128],
                                     gmv[:, 0:1], start=True, stop=True)
                    nc.vector.tensor_mul(acols[:, ct:ct + 1], gnsc[:, ct:ct + 1],
                                         pe_a[:])
                    t2 = statp.tile([128, 1], f32, tag="t2")
                    nc.vector.tensor_mul(t2[:], acols[:, ct:ct + 1], pe_b[:])
                    nc.vector.tensor_tensor(
                        out=bcols[:, ct:ct + 1], in0=gnbc[:, ct:ct + 1],
                        in1=t2[:], op=ALU.subtract)

                # hT = xT * A + B (in place, per channel tile)
                for ct in range(CT):
                    nc.vector.tensor_scalar(
                        ht[:, ct, :], ht[:, ct, :].bitcast(f32),
                        acols[:, ct:ct + 1], bcols[:, ct:ct + 1],
                        op0=ALU.mult, op1=ALU.add)

            phase1a(0)
            phase1b(0)
            phase1a(1)
            for b in range(B_PER):
                ht = ht_tiles[b]
                x_sb = x_tiles[b]

                # ---- projections
                qt = qtp.tile([128, CT, N], f32r, tag="qt")
                kt = ktp.tile([128, CT, N], f32r, tag="kt")
                for dst, w, bias in ((qt, wq, bqc if use_bq else None),
                                     (kt, wk, bkc if use_bk else None)):
                    for ct in range(CT):
                        for ich in range(ICH):
                            pq = pp.tile([128, 512], f32, tag="mm")
                            for cp in range(CT):
                                nc.tensor.matmul(
                                    pq[:],
                                    w[cp][:, ct * 128:(ct + 1) * 128],
                                    ht[:, cp, ich * 512:(ich + 1) * 512],
                                    start=(cp == 0), stop=(cp == CT - 1))
                            dslice = dst[:, ct, ich * 512:(ich + 1) * 512]
                            if bias is not None:
                                nc.scalar.activation(
                                    dslice, pq[:], AF.Identity,
                                    bias=bias[:, ct:ct + 1], scale=1.0)
                            else:
                                nc.scalar.copy(dslice, pq[:])
                v = vp.tile([128, IT, C], f32r, tag="v")
                for it in range(IT):
                    pv = pp.tile([128, 512], f32, tag="mm")
                    for cp in range(CT):
                        nc.tensor.matmul(
                            pv[:], ht[:, cp, it * 128:(it + 1) * 128],
                            wv[cp][:], start=(cp == 0), stop=(cp == CT - 1))
                    nc.vector.tensor_copy(v[:, it, :], pv[:])

                # ---- next batch's phase 1 is emitted here so its transposes
                # and stats chain hide under this batch's attention
                if b + 1 < B_PER:
                    if b + 1 >= 2:
                        phase1a(b + 1)
                    phase1b(b + 1)

                # ---- attention, one 512-token chunk of queries at a time
                for ich in range(ICH):
                    e_t = ep.tile([128, IT, 512], f32r, tag="et")
                    for jt in range(IT):
                        s_ps = pp.tile([128, 512], f32, tag="mm")
                        for cp in range(CT):
                            nc.tensor.matmul(
                                s_ps[:],
                                kt[:, cp, jt * 128:(jt + 1) * 128],
                                qt[:, cp, ich * 512:(ich + 1) * 512],
                                start=(cp == 0), stop=(cp == CT - 1))
                        nc.scalar.activation(e_t[:, jt, :], s_ps[:], AF.Exp,
                                             bias=0.0, scale=SCALE)

                    pl = sp.tile([1, 512], f32, tag="small")
                    for jt in range(IT):
                        nc.tensor.matmul(pl[:], ones_r[:], e_t[:, jt, :],
                                         start=(jt == 0), stop=(jt == IT - 1))
                    lsb = statp.tile([1, 512], f32, tag="lsb")
                    nc.scalar.copy(lsb[:], pl[:])
                    rl = statp.tile([128, 4], f32, tag="rl")
                    for k in range(4):
                        plt = sp.tile([128, 1], f32, tag="small")
                        nc.tensor.matmul(plt[:], lsb[0:1, k * 128:(k + 1) * 128],
                                         onef[0:1, 0:1], start=True, stop=True)
                        nc.vector.reciprocal(rl[:, k:k + 1], plt[:])

                    ut = utp.tile([128, CT, 512], f32r, tag="ut")
                    for ct in range(CT):
                        pu = pp.tile([128, 512], f32, tag="mm")
                        for jt in range(IT):
                            nc.tensor.matmul(
                                pu[:], v[:, jt, ct * 128:(ct + 1) * 128],
                                e_t[:, jt, :],
                                start=(jt == 0), stop=(jt == IT - 1))
                        nc.vector.tensor_copy(ut[:, ct, :], pu[:])

                    for k in range(4):
                        it = ich * 4 + k
                        po = pp.tile([128, 512], f32, tag="mm")
                        for ct in range(CT):
                            nc.tensor.matmul(
                                po[:], ut[:, ct, k * 128:(k + 1) * 128],
                                wo[ct][:], start=(ct == 0), stop=(ct == CT - 1))
                        o_sb = op.tile([128, C], f32, tag="osb")
                        nc.scalar.activation(o_sb[:], po[:], AF.Copy,
                                             bias=0.0, scale=rl[:, k:k + 1])
                        o2 = op.tile([128, C], f32, tag="o2")
                        if use_bo2:
                            nc.vector.tensor_add(o_sb[:], o_sb[:], bo2bc[:])
                        nc.vector.tensor_add(o2[:], o_sb[:], x_sb[:, it, :].bitcast(f32))
                        nc.sync.dma_start(
                            out_d[b, it * 128:(it + 1) * 128, :], o2[:])

    nc.compile()
    return nc


def _host_consts():
    g4 = np.zeros((128, CT * G), np.float32)
    e4 = np.zeros((G, CT * 128), np.float32)
    for ct in range(CT):
        for p in range(128):
            g = ct * 8 + p // 16
            g4[p, ct * G + g] = 1.0
            e4[g, ct * 128 + p] = 1.0
    return g4, e4, np.ones((128, 1), np.float32), np.eye(128, dtype=np.float32)


def kernel(**inputs):
    from concourse import bass_utils

    x = np.ascontiguousarray(np.asarray(inputs["x"], np.float32))
    gn_scale = np.asarray(inputs["gn_scale"], np.float32)
    gn_bias = np.asarray(inputs["gn_bias"], np.float32)
    Wq = np.ascontiguousarray(np.asarray(inputs["Wq"], np.float32))
    Wk = np.ascontiguousarray(np.asarray(inputs["Wk"], np.float32))
    Wv = np.ascontiguousarray(np.asarray(inputs["Wv"], np.float32))
    Wo = np.ascontiguousarray(np.asarray(inputs["Wo"], np.float32))
    bq = np.asarray(inputs["bq"], np.float32)
    bk = np.asarray(inputs["bk"], np.float32)
    bv = np.asarray(inputs["bv"], np.float32)
    bo = np.asarray(inputs["bo"], np.float32)

    B, H, W, Cc = x.shape
    assert (B, H * W, Cc) == (B_TOTAL, N, C)

    bo2 = bv @ Wo + bo
    use_bq = bool(np.any(bq))
    use_bk = bool(np.any(bk))
    use_bo2 = bool(np.any(bo2))

    key = (use_bq, use_bk, use_bo2)
    if key not in _CACHE:
        _CACHE[key] = _build(*key)
    nc = _CACHE[key]

    g4, e4, ones, ident = _host_consts()

    def cols(vec):
        return np.ascontiguousarray(vec.reshape(CT, 128).T)

    base = {
        "wq": Wq, "wk": Wk, "wv": Wv, "wo": Wo,
        "g4": g4, "e4": e4, "ones_in": ones, "ident_in": ident,
        "gnsc": cols(gn_scale), "gnbc": cols(gn_bias),
    }
    if use_bq:
        base["bqc"] = cols(bq)
    if use_bk:
        base["bkc"] = cols(bk)
    if use_bo2:
        base["bo2bc"] = np.ascontiguousarray(
            np.broadcast_to(bo2[None, :], (128, C)))

    x_flat = x.reshape(B_TOTAL, N, C)
    in_maps = []
    for c in range(N_CORES):
        m = dict(base)
        m["xs"] = np.ascontiguousarray(x_flat[c * B_PER:(c + 1) * B_PER])
        in_maps.append(m)

    res = bass_utils.run_bass_kernel_spmd(nc, in_maps,
                                          core_ids=list(range(N_CORES)))
    out = np.concatenate([r["out"] for r in res.results], axis=0)
    return out.reshape(B_TOTAL, H, W, C).astype(np.float32)


# revision 19
# speedup vs baseline: 1.3339x; 1.1855x over previous
"""AttnBlock (GroupNorm -> QKV -> full 1024-token spatial attention -> out-proj
-> residual) for B=32, H=W=32, C=512 on 8 Trainium2 NeuronCores.

Sharding: data-parallel over batch (4 batch elements per core). Everything on
one core is a single Bass/Tile program:

  per batch element b (activations as [tokens=1024, C=512]):
    x      -> PE-transpose -> xT [C-part, tok]  (rounded to f32r)
    stats: bn_stats per channel + tiny G-matmul for per-group mean/E[x^2]
    hT = xT * A + B in place (A,B per-channel from group stats; f32r)
    QT = Wq^T hT (+bq), KT = Wk^T hT (+bk)    [C-part, tok]  f32r
    V  = hT^T Wv                              [tok-part, C]  f32r
    per 512-token chunk i:
      ET[j,i] = exp(scale * KT^T QT)          [tok_j-part, i] f32r
      l[i] = ones^T ET;  rl = 1/l transposed to partitions via K=1 matmuls
      UT = V^T ET (unnormalized PV)           [C-part, i]
      out = (UT^T Wo) * rl + bo2 + x          [tok-part, C]

All big matmuls run in float32r (TF32-like, full PE rate, ~1e-4 rel rounding),
K=128 per accumulation step. Tiny matmuls (group reduce/expand, l-transpose)
run in plain fp32. bv/bo are folded into bo2 = bv @ Wo + bo on host (softmax
rows sum to 1, so +bv on V becomes +bv on PV).
"""

import math

import numpy as np

B_TOTAL = 32
N_CORES = 8
B_PER = B_TOTAL // N_CORES
N = 1024
C = 512
G = 32
CT = 4     # channel tiles of 128
IT = 8     # token tiles of 128
ICH = 2    # token chunks of 512
EPS = 1e-6
SCALE = 1.0 / math.sqrt(C)

_CACHE = {}


def _build(use_bq, use_bk, use_bo2):
    merged = not (use_bq or use_bk)
    import concourse.tile as tile
    from concourse import bacc, mybir
    f32 = mybir.dt.float32
    f32r = mybir.dt.float32r
    AF = mybir.ActivationFunctionType
    ALU = mybir.AluOpType

    nc = bacc.Bacc("TRN2", target_bir_lowering=False, debug=False,
                   num_devices=N_CORES)

    xs_d = nc.dram_tensor("xs", [B_PER, N, C], f32r, kind="ExternalInput").ap()
    w_names = ("wm", "wvo") if merged else ("wq", "wk", "wv", "wo")
    w_d = {
        name: nc.dram_tensor(name, [C, C], f32r, kind="ExternalInput").ap()
        for name in w_names
    }
    g4_d = nc.dram_tensor("g4", [128, CT * G], f32, kind="ExternalInput").ap()
    e4_d = nc.dram_tensor("e4", [G, CT * 128], f32, kind="ExternalInput").ap()
    ones_d = nc.dram_tensor("ones_in", [128, 1], f32r, kind="ExternalInput").ap()
    id_d = nc.dram_tensor("ident_in", [128, 128], f32r, kind="ExternalInput").ap()
    gns_d = nc.dram_tensor("gnsc", [128, CT], f32, kind="ExternalInput").ap()
    gnb_d = nc.dram_tensor("gnbc", [128, CT], f32, kind="ExternalInput").ap()
    bq_d = nc.dram_tensor("bqc", [128, CT], f32, kind="ExternalInput").ap() if use_bq else None
    bk_d = nc.dram_tensor("bkc", [128, CT], f32, kind="ExternalInput").ap() if use_bk else None
    bo2_d = nc.dram_tensor("bo2bc", [128, C], f32, kind="ExternalInput").ap() if use_bo2 else None
    out_d = nc.dram_tensor("out", [B_PER, N, C], f32, kind="ExternalOutput").ap()

    with tile.TileContext(nc) as tc:
        with (
            tc.tile_pool(name="consts", bufs=1) as consts,
            tc.tile_pool(name="xp", bufs=2) as xp,
            tc.tile_pool(name="htp", bufs=2) as htp,
            tc.tile_pool(name="qtp", bufs=1) as qtp,
            tc.tile_pool(name="ktp", bufs=1) as ktp,
            tc.tile_pool(name="vp", bufs=1) as vp,
            tc.tile_pool(name="ep", bufs=1) as ep,
            tc.tile_pool(name="utp", bufs=2) as utp,
            tc.tile_pool(name="op", bufs=3) as op,
            tc.tile_pool(name="statp", bufs=2) as statp,
            tc.tile_pool(name="pp", bufs=6, space="PSUM") as pp,
            tc.tile_pool(name="sp", bufs=2, space="PSUM") as sp,
        ):
            # ---- first batch's x load goes ahead of the weight DMAs so the
            # PE transposes can start while weights stream in
            ident = consts.tile([128, 128], f32r)
            nc.sync.dma_start(ident[:], id_d[:])
            x_tiles = {}
            x_tiles[0] = xp.tile([128, IT, C], f32r, name="x0", tag="x")
            for it in range(IT):
                nc.sync.dma_start(x_tiles[0][:, it, :],
                                  xs_d[0, it * 128:(it + 1) * 128, :])
            # warm up the PE clock (HAM) while the first x tiles stream in
            wu = pp.tile([128, 512], f32r, name="wu", tag="mm")
            for i in range(10):
                nc.tensor.transpose(wu[:, (i % 4) * 128:(i % 4 + 1) * 128],
                                    ident[:], ident[:])

            # ---- small consts first (needed by the batch-0 stats chain),
            # then weights in consumption order wq -> wk -> wv -> wo
            g4 = consts.tile([128, CT * G], f32)
            nc.gpsimd.dma_start(g4[:], g4_d[:])
            e4 = consts.tile([G, CT * 128], f32)
            nc.gpsimd.dma_start(e4[:], e4_d[:])
            ones_r = consts.tile([128, 1], f32r)
            nc.gpsimd.dma_start(ones_r[:], ones_d[:])
            gnsc = consts.tile([128, CT], f32)
            nc.gpsimd.dma_start(gnsc[:], gns_d[:])
            gnbc = consts.tile([128, CT], f32)
            nc.gpsimd.dma_start(gnbc[:], gnb_d[:])
            if use_bq:
                bqc = consts.tile([128, CT], f32)
                nc.gpsimd.dma_start(bqc[:], bq_d[:])
            if use_bk:
                bkc = consts.tile([128, CT], f32)
                nc.gpsimd.dma_start(bkc[:], bk_d[:])
            if use_bo2:
                bo2bc = consts.tile([128, C], f32)
                nc.gpsimd.dma_start(bo2bc[:], bo2_d[:])
            onef = consts.tile([128, 1], f32)
            nc.vector.memset(onef[:], 1.0)
            eps32 = consts.tile([G, 1], f32)
            nc.vector.memset(eps32[:], EPS)
            wt = {
                nm: [consts.tile([128, C], f32r, name=f"{nm}{i}", tag=f"{nm}{i}")
                     for i in range(CT)]
                for nm in w_names
            }
            for nm in w_names:
                for i in range(CT):
                    nc.gpsimd.dma_start(wt[nm][i][:],
                                        w_d[nm][i * 128:(i + 1) * 128, :])

            ht_tiles = {}

            def load_x(b):
                if b not in x_tiles:
                    x_sb = xp.tile([128, IT, C], f32r, name="x_sb", tag="x")
                    for it in range(IT):
                        nc.sync.dma_start(x_sb[:, it, :],
                                          xs_d[b, it * 128:(it + 1) * 128, :])
                    x_tiles[b] = x_sb
                return x_tiles[b]

            def phase1a(b):
                # transpose x -> ht (f32r-rounded), token-group major so the
                # first transposes only need the first half of x
                x_sb = load_x(b)
                ht = htp.tile([128, CT, N], f32r, name="ht", tag="ht")
                ht_tiles[b] = ht
                for ig in range(2):
                    for ct in range(CT):
                        ptr = pp.tile([128, 512], f32r, name="ptr", tag="mm")
                        for k in range(4):
                            it = ig * 4 + k
                            nc.tensor.transpose(
                                ptr[:, k * 128:(k + 1) * 128],
                                x_sb[:, it, ct * 128:(ct + 1) * 128],
                                ident[:])
                        nc.scalar.copy(ht[:, ct, ig * 512:(ig + 1) * 512], ptr[:])

            def phase1b(b):
                # groupnorm stats + in-place affine on ht
                ht = ht_tiles[b]
                # per-channel stats over the 1024 tokens
                stats = statp.tile([128, CT, 2, 6], f32, name="stats", tag="stats")
                mvt = statp.tile([128, CT, 2], f32, name="mvt", tag="mvt")
                ms = statp.tile([128, CT, 2], f32, name="ms", tag="ms")
                for ct in range(CT):
                    for h in range(2):
                        nc.vector.bn_stats(
                            stats[:, ct, h, :],
                            ht[:, ct, h * 512:(h + 1) * 512].bitcast(f32))
                    nc.vector.bn_aggr(mvt[:, ct, :], stats[:, ct, :, :])
                    nc.vector.tensor_copy(ms[:, ct, 0:1], mvt[:, ct, 0:1])
                    t1 = statp.tile([128, 1], f32, tag="t1")
                    nc.vector.tensor_mul(t1[:], mvt[:, ct, 0:1], mvt[:, ct, 0:1])
                    nc.vector.tensor_add(ms[:, ct, 1:2], mvt[:, ct, 1:2], t1[:])

                # ---- group reduce: [32, (mean, E[x^2])] = G4^T @ ms / 16
                pg = sp.tile([G, 2], f32, tag="small")
                for ct in range(CT):
                    nc.tensor.matmul(pg[:], g4[:, ct * G:(ct + 1) * G],
                                     ms[:, ct, :],
                                     start=(ct == 0), stop=(ct == CT - 1))
                gmv = statp.tile([G, 2], f32, tag="gmv")
                nc.vector.tensor_scalar_mul(gmv[:], pg[:], 1.0 / 16.0)
                m2 = statp.tile([G, 1], f32, tag="m2")
                nc.vector.tensor_mul(m2[:], gmv[:, 0:1], gmv[:, 0:1])
                var32 = statp.tile([G, 1], f32, tag="var32")
                nc.vector.tensor_tensor(
                    out=var32[:], in0=gmv[:, 1:2], in1=m2[:], op=ALU.subtract)
                std32 = statp.tile([G, 1], f32, tag="std32")
                nc.scalar.activation(std32[:], var32[:], AF.Sqrt,
                                     bias=eps32[:], scale=1.0)
                rstd32 = statp.tile([G, 1], f32, tag="rstd32")
                nc.vector.reciprocal(rstd32[:], std32[:])

                # ---- expand group stats to channels; A/B affine coefs
                acols = statp.tile([128, CT], f32, tag="acols")
                bcols = statp.tile([128, CT], f32, tag="bcols")
                for ct in range(CT):
                    pe_a = sp.tile([128, 1], f32, tag="small")
                    nc.tensor.matmul(pe_a[:], e4[:, ct * 128:(ct + 1) * 128],
                                     rstd32[:], start=True, stop=True)
                    pe_b = sp.tile([128, 1], f32, tag="small")
                    nc.tensor.matmul(pe_b[:], e4[:, ct * 128:(ct + 1) * 128],
                                     gmv[:, 0:1], start=True, stop=True)
                    nc.vector.tensor_mul(acols[:, ct:ct + 1], gnsc[:, ct:ct + 1],
                                         pe_a[:])
                    t2 = statp.tile([128, 1], f32, tag="t2")
                    nc.vector.tensor_mul(t2[:], acols[:, ct:ct + 1], pe_b[:])
                    nc.vector.tensor_tensor(
                        out=bcols[:, ct:ct + 1], in0=gnbc[:, ct:ct + 1],
                        in1=t2[:], op=ALU.subtract)

                # hT = xT * A + B (in place, per channel tile)
                for ct in range(CT):
                    nc.vector.tensor_scalar(
                        ht[:, ct, :], ht[:, ct, :].bitcast(f32),
                        acols[:, ct:ct + 1], bcols[:, ct:ct + 1],
                        op0=ALU.mult, op1=ALU.add)

            phase1a(0)
            phase1b(0)
            phase1a(1)
            for b in range(B_PER):
                ht = ht_tiles[b]
                x_sb = x_tiles[b]

                # ---- projections
                # merged: kt = (Wq Wk^T)^T h^T; S^T = kt^T ht needs no q.
                #         v = h (Wv Wo); the out-projection becomes a transpose.
                if merged:
                    proj_list = [("kt", wt["wm"]), ("v", wt["wvo"])]
                else:
                    proj_list = [("qt", wt["wq"]), ("kt", wt["wk"]),
                                 ("v", wt["wv"])]
                qt = None
                for dname, w in proj_list:
                    if dname == "v":
                        v = vp.tile([128, IT, C], f32r, tag="v")
                        for it in range(IT):
                            pv = pp.tile([128, 512], f32, tag="mm")
                            for cp in range(CT):
                                nc.tensor.matmul(
                                    pv[:], ht[:, cp, it * 128:(it + 1) * 128],
                                    w[cp][:], start=(cp == 0),
                                    stop=(cp == CT - 1))
                            nc.vector.tensor_copy(v[:, it, :], pv[:])
                        continue
                    dst = (qtp if dname == "qt" else ktp).tile(
                        [128, CT, N], f32r, name=dname, tag=dname)
                    if dname == "qt":
                        qt = dst
                        bias = bqc if use_bq else None
                    else:
                        kt = dst
                        bias = bkc if use_bk else None
                    for ct in range(CT):
                        for ich in range(ICH):
                            pq = pp.tile([128, 512], f32, tag="mm")
                            for cp in range(CT):
                                nc.tensor.matmul(
                                    pq[:],
                                    w[cp][:, ct * 128:(ct + 1) * 128],
                                    ht[:, cp, ich * 512:(ich + 1) * 512],
                                    start=(cp == 0), stop=(cp == CT - 1))
                            dslice = dst[:, ct, ich * 512:(ich + 1) * 512]
                            if bias is not None:
                                nc.scalar.activation(
                                    dslice, pq[:], AF.Identity,
                                    bias=bias[:, ct:ct + 1], scale=1.0)
                            else:
                                nc.scalar.copy(dslice, pq[:])

                # ---- next batch's phase 1 is emitted here so its transposes
                # and stats chain hide under this batch's attention
                if b + 1 < B_PER:
                    if b + 1 >= 2:
                        phase1a(b + 1)
                    phase1b(b + 1)

                # ---- attention, one 512-token chunk of queries at a time
                for ich in range(ICH):
                    e_t = ep.tile([128, IT, 512], f32r, tag="et")
                    for jt in range(IT):
                        s_ps = pp.tile([128, 512], f32, tag="mm")
                        s_rhs = ht if merged else qt
                        for cp in range(CT):
                            nc.tensor.matmul(
                                s_ps[:],
                                kt[:, cp, jt * 128:(jt + 1) * 128],
                                s_rhs[:, cp, ich * 512:(ich + 1) * 512],
                                start=(cp == 0), stop=(cp == CT - 1))
                        nc.scalar.activation(e_t[:, jt, :], s_ps[:], AF.Exp,
                                             bias=0.0, scale=SCALE)

                    pl = sp.tile([1, 512], f32, tag="small")
                    for jt in range(IT):
                        nc.tensor.matmul(pl[:], ones_r[:], e_t[:, jt, :],
                                         start=(jt == 0), stop=(jt == IT - 1))
                    lsb = statp.tile([1, 512], f32, tag="lsb")
                    nc.scalar.copy(lsb[:], pl[:])
                    rl = statp.tile([128, 4], f32, tag="rl")
                    for k in range(4):
                        plt = sp.tile([128, 1], f32, tag="small")
                        nc.tensor.matmul(plt[:], lsb[0:1, k * 128:(k + 1) * 128],
                                         onef[0:1, 0:1], start=True, stop=True)
                        nc.vector.reciprocal(rl[:, k:k + 1], plt[:])

                    ut = utp.tile([128, CT, 512], f32r, tag="ut")
                    for ct in range(CT):
                        pu = pp.tile([128, 512], f32, tag="mm")
                        for jt in range(IT):
                            nc.tensor.matmul(
                                pu[:], v[:, jt, ct * 128:(ct + 1) * 128],
                                e_t[:, jt, :],
                                start=(jt == 0), stop=(jt == IT - 1))
                        nc.vector.tensor_copy(ut[:, ct, :], pu[:])

                    for k in range(4):
                        it = ich * 4 + k
                        if merged:
                            po = pp.tile([128, 512], f32r, name="po", tag="mm")
                            for ct in range(CT):
                                nc.tensor.transpose(
                                    po[:, ct * 128:(ct + 1) * 128],
                                    ut[:, ct, k * 128:(k + 1) * 128],
                                    ident[:])
                        else:
                            po = pp.tile([128, 512], f32, name="po", tag="mm")
                            for ct in range(CT):
                                nc.tensor.matmul(
                                    po[:], ut[:, ct, k * 128:(k + 1) * 128],
                                    wt["wo"][ct][:], start=(ct == 0),
                                    stop=(ct == CT - 1))
                        o_sb = op.tile([128, C], f32, tag="osb")
                        nc.scalar.activation(o_sb[:], po[:], AF.Copy,
                                             bias=0.0, scale=rl[:, k:k + 1])
                        o2 = op.tile([128, C], f32, tag="o2")
                        if use_bo2:
                            nc.vector.tensor_add(o_sb[:], o_sb[:], bo2bc[:])
                        nc.vector.tensor_add(o2[:], o_sb[:], x_sb[:, it, :].bitcast(f32))
                        nc.sync.dma_start(
                            out_d[b, it * 128:(it + 1) * 128, :], o2[:])

    nc.compile()
    return nc


def _host_consts():
    g4 = np.zeros((128, CT * G), np.float32)
    e4 = np.zeros((G, CT * 128), np.float32)
    for ct in range(CT):
        for p in range(128):
            g = ct * 8 + p // 16
            g4[p, ct * G + g] = 1.0
            e4[g, ct * 128 + p] = 1.0
    return g4, e4, np.ones((128, 1), np.float32), np.eye(128, dtype=np.float32)


def kernel(**inputs):
    from concourse import bass_utils

    x = np.ascontiguousarray(np.asarray(inputs["x"], np.float32))
    gn_scale = np.asarray(inputs["gn_scale"], np.float32)
    gn_bias = np.asarray(inputs["gn_bias"], np.float32)
    Wq = np.ascontiguousarray(np.asarray(inputs["Wq"], np.float32))
    Wk = np.ascontiguousarray(np.asarray(inputs["Wk"], np.float32))
    Wv = np.ascontiguousarray(np.asarray(inputs["Wv"], np.float32))
    Wo = np.ascontiguousarray(np.asarray(inputs["Wo"], np.float32))
    bq = np.asarray(inputs["bq"], np.float32)
    bk = np.asarray(inputs["bk"], np.float32)
    bv = np.asarray(inputs["bv"], np.float32)
    bo = np.asarray(inputs["bo"], np.float32)

    B, H, W, Cc = x.shape
    assert (B, H * W, Cc) == (B_TOTAL, N, C)

    bo2 = bv @ Wo + bo
    use_bq = bool(np.any(bq))
    use_bk = bool(np.any(bk))
    use_bo2 = bool(np.any(bo2))

    key = (use_bq, use_bk, use_bo2)
    if key not in _CACHE:
        _CACHE[key] = _build(*key)
    nc = _CACHE[key]

    g4, e4, ones, ident = _host_consts()

    def cols(vec):
        return np.ascontiguousarray(vec.reshape(CT, 128).T)

    base = {
        "g4": g4, "e4": e4, "ones_in": ones, "ident_in": ident,
        "gnsc": cols(gn_scale), "gnbc": cols(gn_bias),
    }
    if not (use_bq or use_bk):
        base["wm"] = np.ascontiguousarray(
            (Wk.astype(np.float64) @ Wq.T.astype(np.float64)).astype(np.float32))
        base["wvo"] = np.ascontiguousarray(
            (Wv.astype(np.float64) @ Wo.astype(np.float64)).astype(np.float32))
    else:
        base.update({"wq": Wq, "wk": Wk, "wv": Wv, "wo": Wo})
    if use_bq:
        base["bqc"] = cols(bq)
    if use_bk:
        base["bkc"] = cols(bk)
    if use_bo2:
        base["bo2bc"] = np.ascontiguousarray(
            np.broadcast_to(bo2[None, :], (128, C)))

    x_flat = x.reshape(B_TOTAL, N, C)
    in_maps = []
    for c in range(N_CORES):
        m = dict(base)
        m["xs"] = np.ascontiguousarray(x_flat[c * B_PER:(c + 1) * B_PER])
        in_maps.append(m)

    res = bass_utils.run_bass_kernel_spmd(nc, in_maps,
                                          core_ids=list(range(N_CORES)))
    out = np.concatenate([r["out"] for r in res.results], axis=0)
    return out.reshape(B_TOTAL, H, W, C).astype(np.float32)


# revision 28
# speedup vs baseline: 1.3543x; 1.0153x over previous
"""AttnBlock (GroupNorm -> QKV -> full 1024-token spatial attention -> out-proj
-> residual) for B=32, H=W=32, C=512 on 8 Trainium2 NeuronCores.

Sharding: data-parallel over batch (4 batch elements per core). Everything on
one core is a single Bass/Tile program:

  per batch element b (activations as [tokens=1024, C=512]):
    x      -> PE-transpose -> xT [C-part, tok]  (rounded to f32r)
    stats: bn_stats per channel + tiny G-matmul for per-group mean/E[x^2]
    hT = xT * A + B in place (A,B per-channel from group stats; f32r)
    QT = Wq^T hT (+bq), KT = Wk^T hT (+bk)    [C-part, tok]  f32r
    V  = hT^T Wv                              [tok-part, C]  f32r
    per 512-token chunk i:
      ET[j,i] = exp(scale * KT^T QT)          [tok_j-part, i] f32r
      l[i] = ones^T ET;  rl = 1/l transposed to partitions via K=1 matmuls
      UT = V^T ET (unnormalized PV)           [C-part, i]
      out = (UT^T Wo) * rl + bo2 + x          [tok-part, C]

All big matmuls run in float32r (TF32-like, full PE rate, ~1e-4 rel rounding),
K=128 per accumulation step. Tiny matmuls (group reduce/expand, l-transpose)
run in plain fp32. bv/bo are folded into bo2 = bv @ Wo + bo on host (softmax
rows sum to 1, so +bv on V becomes +bv on PV).
"""

import math

import numpy as np

B_TOTAL = 32
N_CORES = 8
B_PER = B_TOTAL // N_CORES
N = 1024
C = 512
G = 32
CT = 4     # channel tiles of 128
IT = 8     # token tiles of 128
ICH = 2    # token chunks of 512
EPS = 1e-6
SCALE = 1.0 / math.sqrt(C)

_CACHE = {}


def _build(use_bq, use_bk, use_bo2):
    merged = not (use_bq or use_bk)
    import concourse.tile as tile
    from concourse import bacc, mybir
    f32 = mybir.dt.float32
    f32r = mybir.dt.float32r
    AF = mybir.ActivationFunctionType
    ALU = mybir.AluOpType

    nc = bacc.Bacc("TRN2", target_bir_lowering=False, debug=False,
                   num_devices=N_CORES)

    xs_d = nc.dram_tensor("xs", [B_PER, N, C], f32r, kind="ExternalInput").ap()
    w_names = ("wm", "wvo") if merged else ("wq", "wk", "wv", "wo")
    w_d = {
        name: nc.dram_tensor(name, [C, C], f32r, kind="ExternalInput").ap()
        for name in w_names
    }
    g4_d = nc.dram_tensor("g4", [128, CT * G], f32, kind="ExternalInput").ap()
    e4_d = nc.dram_tensor("e4", [G, CT * 128], f32, kind="ExternalInput").ap()
    ones_d = nc.dram_tensor("ones_in", [128, 1], f32r, kind="ExternalInput").ap()
    id_d = nc.dram_tensor("ident_in", [128, 128], f32r, kind="ExternalInput").ap()
    gns_d = nc.dram_tensor("gnsc", [128, CT], f32, kind="ExternalInput").ap()
    gnb_d = nc.dram_tensor("gnbc", [128, CT], f32, kind="ExternalInput").ap()
    bq_d = nc.dram_tensor("bqc", [128, CT], f32, kind="ExternalInput").ap() if use_bq else None
    bk_d = nc.dram_tensor("bkc", [128, CT], f32, kind="ExternalInput").ap() if use_bk else None
    bo2_d = nc.dram_tensor("bo2bc", [128, C], f32, kind="ExternalInput").ap() if use_bo2 else None
    out_d = nc.dram_tensor("out", [B_PER, N, C], f32, kind="ExternalOutput").ap()

    with tile.TileContext(nc) as tc:
        with (
            tc.tile_pool(name="consts", bufs=1) as consts,
            tc.tile_pool(name="xp", bufs=2) as xp,
            tc.tile_pool(name="htp", bufs=2) as htp,
            tc.tile_pool(name="qtp", bufs=1) as qtp,
            tc.tile_pool(name="ktp", bufs=1) as ktp,
            tc.tile_pool(name="vp", bufs=1) as vp,
            tc.tile_pool(name="ep", bufs=2) as ep,
            tc.tile_pool(name="utp", bufs=2) as utp,
            tc.tile_pool(name="op", bufs=3) as op,
            tc.tile_pool(name="statp", bufs=2) as statp,
            tc.tile_pool(name="pp", bufs=6, space="PSUM") as pp,
            tc.tile_pool(name="sp", bufs=2, space="PSUM") as sp,
        ):
            # ---- first batch's x load goes ahead of the weight DMAs so the
            # PE transposes can start while weights stream in
            ident = consts.tile([128, 128], f32r)
            nc.sync.dma_start(ident[:], id_d[:])
            x_tiles = {}
            x_tiles[0] = xp.tile([128, IT, C], f32r, name="x0", tag="x")
            for it in range(IT):
                nc.sync.dma_start(x_tiles[0][:, it, :],
                                  xs_d[0, it * 128:(it + 1) * 128, :])


            # ---- small consts first (needed by the batch-0 stats chain),
            # then weights in consumption order wq -> wk -> wv -> wo
            g4 = consts.tile([128, CT * G], f32)
            nc.gpsimd.dma_start(g4[:], g4_d[:])
            e4 = consts.tile([G, CT * 128], f32)
            nc.gpsimd.dma_start(e4[:], e4_d[:])
            ones_r = consts.tile([128, 1], f32r)
            nc.gpsimd.dma_start(ones_r[:], ones_d[:])
            gnsc = consts.tile([128, CT], f32)
            nc.gpsimd.dma_start(gnsc[:], gns_d[:])
            gnbc = consts.tile([128, CT], f32)
            nc.gpsimd.dma_start(gnbc[:], gnb_d[:])
            if use_bq:
                bqc = consts.tile([128, CT], f32)
                nc.gpsimd.dma_start(bqc[:], bq_d[:])
            if use_bk:
                bkc = consts.tile([128, CT], f32)
                nc.gpsimd.dma_start(bkc[:], bk_d[:])
            if use_bo2:
                bo2bc = consts.tile([128, C], f32)
                nc.gpsimd.dma_start(bo2bc[:], bo2_d[:])
            onef = consts.tile([128, 1], f32)
            nc.vector.memset(onef[:], 1.0)
            eps32 = consts.tile([G, 1], f32)
            nc.vector.memset(eps32[:], EPS)

            # batch-1 x right behind x0, ahead of the weights, so batch-1
            # transposes can fill the batch-0 stats-chain bubble
            x_tiles[1] = xp.tile([128, IT, C], f32r, name="x1", tag="x")
            for it in range(IT):
                nc.sync.dma_start(x_tiles[1][:, it, :],
                                  xs_d[1, it * 128:(it + 1) * 128, :])

            wt = {
                nm: [consts.tile([128, C], f32r, name=f"{nm}{i}", tag=f"{nm}{i}")
                     for i in range(CT)]
                for nm in w_names
            }
            for nm in w_names:
                for i in range(CT):
                    nc.sync.dma_start(wt[nm][i][:],
                                       w_d[nm][i * 128:(i + 1) * 128, :])

            ht_tiles = {}

            def load_x(b):
                if b not in x_tiles:
                    x_sb = xp.tile([128, IT, C], f32r, name="x_sb", tag="x")
                    for it in range(IT):
                        nc.sync.dma_start(x_sb[:, it, :],
                                          xs_d[b, it * 128:(it + 1) * 128, :])
                    x_tiles[b] = x_sb
                return x_tiles[b]

            def phase1a(b):
                # transpose x -> ht (f32r-rounded), token-group major so the
                # first transposes only need the first half of x
                x_sb = load_x(b)
                ht = htp.tile([128, CT, N], f32r, name="ht", tag="ht")
                ht_tiles[b] = ht
                for ig in range(2):
                    for ct in range(CT):
                        ptr = pp.tile([128, 512], f32r, name="ptr", tag="mm")
                        for k in range(4):
                            it = ig * 4 + k
                            nc.tensor.transpose(
                                ptr[:, k * 128:(k + 1) * 128],
                                x_sb[:, it, ct * 128:(ct + 1) * 128],
                                ident[:])
                        nc.scalar.copy(ht[:, ct, ig * 512:(ig + 1) * 512], ptr[:])

            def phase1b(b):
                # groupnorm stats + in-place affine on ht
                ht = ht_tiles[b]
                # per-channel stats over the 1024 tokens
                stats = statp.tile([128, CT, 2, 6], f32, name="stats", tag="stats")
                mvt = statp.tile([128, CT, 2], f32, name="mvt", tag="mvt")
                ms = statp.tile([128, CT, 2], f32, name="ms", tag="ms")
                for ct in range(CT):
                    for h in range(2):
                        nc.vector.bn_stats(
                            stats[:, ct, h, :],
                            ht[:, ct, h * 512:(h + 1) * 512].bitcast(f32))
                    nc.vector.bn_aggr(mvt[:, ct, :], stats[:, ct, :, :])
                    nc.vector.tensor_copy(ms[:, ct, 0:1], mvt[:, ct, 0:1])
                    t1 = statp.tile([128, 1], f32, tag="t1")
                    nc.vector.tensor_mul(t1[:], mvt[:, ct, 0:1], mvt[:, ct, 0:1])
                    nc.vector.tensor_add(ms[:, ct, 1:2], mvt[:, ct, 1:2], t1[:])

                # ---- group reduce: [32, (mean, E[x^2])] = G4^T @ ms / 16
                pg = sp.tile([G, 2], f32, tag="small")
                for ct in range(CT):
                    nc.tensor.matmul(pg[:], g4[:, ct * G:(ct + 1) * G],
                                     ms[:, ct, :],
                                     start=(ct == 0), stop=(ct == CT - 1))
                gmv = statp.tile([G, 2], f32, tag="gmv")
                nc.vector.tensor_scalar_mul(gmv[:], pg[:], 1.0 / 16.0)
                m2 = statp.tile([G, 1], f32, tag="m2")
                nc.vector.tensor_mul(m2[:], gmv[:, 0:1], gmv[:, 0:1])
                var32 = statp.tile([G, 1], f32, tag="var32")
                nc.vector.tensor_tensor(
                    out=var32[:], in0=gmv[:, 1:2], in1=m2[:], op=ALU.subtract)
                std32 = statp.tile([G, 1], f32, tag="std32")
                nc.scalar.activation(std32[:], var32[:], AF.Sqrt,
                                     bias=eps32[:], scale=1.0)
                rstd32 = statp.tile([G, 1], f32, tag="rstd32")
                nc.vector.reciprocal(rstd32[:], std32[:])

                # ---- expand group stats to channels; A/B affine coefs
                acols = statp.tile([128, CT], f32, tag="acols")
                bcols = statp.tile([128, CT], f32, tag="bcols")
                for ct in range(CT):
                    pe_a = sp.tile([128, 1], f32, tag="small")
                    nc.tensor.matmul(pe_a[:], e4[:, ct * 128:(ct + 1) * 128],
                                     rstd32[:], start=True, stop=True)
                    pe_b = sp.tile([128, 1], f32, tag="small")
                    nc.tensor.matmul(pe_b[:], e4[:, ct * 128:(ct + 1) * 128],
                                     gmv[:, 0:1], start=True, stop=True)
                    nc.vector.tensor_mul(acols[:, ct:ct + 1], gnsc[:, ct:ct + 1],
                                         pe_a[:])
                    t2 = statp.tile([128, 1], f32, tag="t2")
                    nc.vector.tensor_mul(t2[:], acols[:, ct:ct + 1], pe_b[:])
                    nc.vector.tensor_tensor(
                        out=bcols[:, ct:ct + 1], in0=gnbc[:, ct:ct + 1],
                        in1=t2[:], op=ALU.subtract)

                # hT = xT * A + B (in place, per channel tile)
                for ct in range(CT):
                    nc.vector.tensor_scalar(
                        ht[:, ct, :], ht[:, ct, :].bitcast(f32),
                        acols[:, ct:ct + 1], bcols[:, ct:ct + 1],
                        op0=ALU.mult, op1=ALU.add)

            phase1a(0)
            phase1b(0)
            phase1a(1)
            for b in range(B_PER):
                ht = ht_tiles[b]
                x_sb = x_tiles[b]

                # ---- projections
                # merged: kt = (Wq Wk^T)^T h^T; S^T = kt^T ht needs no q.
                #         v = h (Wv Wo); the out-projection becomes a transpose.
                if merged:
                    proj_list = [("kt", wt["wm"]), ("v", wt["wvo"])]
                else:
                    proj_list = [("qt", wt["wq"]), ("kt", wt["wk"]),
                                 ("v", wt["wv"])]
                qt = None
                for dname, w in proj_list:
                    if dname == "v":
                        v = vp.tile([128, IT, C], f32r, tag="v")
                        for it in range(IT):
                            pv = pp.tile([128, 512], f32, tag="mm")
                            for cp in range(CT):
                                nc.tensor.matmul(
                                    pv[:], ht[:, cp, it * 128:(it + 1) * 128],
                                    w[cp][:], start=(cp == 0),
                                    stop=(cp == CT - 1))
                            nc.vector.tensor_copy(v[:, it, :], pv[:])
                        continue
                    dst = (qtp if dname == "qt" else ktp).tile(
                        [128, CT, N], f32r, name=dname, tag=dname)
                    if dname == "qt":
                        qt = dst
                        bias = bqc if use_bq else None
                    else:
                        kt = dst
                        bias = bkc if use_bk else None
                    for ct in range(CT):
                        for ich in range(ICH):
                            pq = pp.tile([128, 512], f32, tag="mm")
                            for cp in range(CT):
                                nc.tensor.matmul(
                                    pq[:],
                                    w[cp][:, ct * 128:(ct + 1) * 128],
                                    ht[:, cp, ich * 512:(ich + 1) * 512],
                                    start=(cp == 0), stop=(cp == CT - 1))
                            dslice = dst[:, ct, ich * 512:(ich + 1) * 512]
                            if bias is not None:
                                nc.scalar.activation(
                                    dslice, pq[:], AF.Identity,
                                    bias=bias[:, ct:ct + 1], scale=1.0)
                            else:
                                nc.scalar.copy(dslice, pq[:])

                # ---- next batch's phase 1 is emitted here so its transposes
                # and stats chain hide under this batch's attention
                if b + 1 < B_PER:
                    if b + 1 >= 2:
                        phase1a(b + 1)
                    phase1b(b + 1)

                # ---- attention, one 512-token chunk of queries at a time
                for ich in range(ICH):
                    e_t = ep.tile([128, IT, 512], f32r, tag="et")
                    for jt in range(IT):
                        s_ps = pp.tile([128, 512], f32, tag="mm")
                        s_rhs = ht if merged else qt
                        for cp in range(CT):
                            nc.tensor.matmul(
                                s_ps[:],
                                kt[:, cp, jt * 128:(jt + 1) * 128],
                                s_rhs[:, cp, ich * 512:(ich + 1) * 512],
                                start=(cp == 0), stop=(cp == CT - 1))
                        nc.scalar.activation(e_t[:, jt, :], s_ps[:], AF.Exp,
                                             bias=0.0, scale=SCALE)

                    pl = sp.tile([1, 512], f32, tag="small")
                    for jt in range(IT):
                        nc.tensor.matmul(pl[:], ones_r[:], e_t[:, jt, :],
                                         start=(jt == 0), stop=(jt == IT - 1))
                    lsb = statp.tile([1, 512], f32, tag="lsb")
                    nc.scalar.copy(lsb[:], pl[:])
                    rl = statp.tile([128, 4], f32, tag="rl")
                    for k in range(4):
                        plt = sp.tile([128, 1], f32, tag="small")
                        nc.tensor.matmul(plt[:], lsb[0:1, k * 128:(k + 1) * 128],
                                         onef[0:1, 0:1], start=True, stop=True)
                        nc.vector.reciprocal(rl[:, k:k + 1], plt[:])

                    ut = utp.tile([128, CT, 512], f32r, tag="ut")
                    for ct in range(CT):
                        pu = pp.tile([128, 512], f32, tag="mm")
                        for jt in range(IT):
                            nc.tensor.matmul(
                                pu[:], v[:, jt, ct * 128:(ct + 1) * 128],
                                e_t[:, jt, :],
                                start=(jt == 0), stop=(jt == IT - 1))
                        if ct % 2 == 0:
                            nc.vector.tensor_copy(ut[:, ct, :], pu[:])
                        else:
                            nc.scalar.copy(ut[:, ct, :], pu[:])

                    for k in range(4):
                        it = ich * 4 + k
                        if merged:
                            po = pp.tile([128, 512], f32r, name="po", tag="mm")
                            for ct in range(CT):
                                nc.tensor.transpose(
                                    po[:, ct * 128:(ct + 1) * 128],
                                    ut[:, ct, k * 128:(k + 1) * 128],
                                    ident[:])
                        else:
                            po = pp.tile([128, 512], f32, name="po", tag="mm")
                            for ct in range(CT):
                                nc.tensor.matmul(
                                    po[:], ut[:, ct, k * 128:(k + 1) * 128],
                                    wt["wo"][ct][:], start=(ct == 0),
                                    stop=(ct == CT - 1))
                        o_sb = op.tile([128, C], f32, tag="osb")
                        nc.scalar.activation(o_sb[:], po[:], AF.Copy,
                                             bias=0.0, scale=rl[:, k:k + 1])
                        o2 = op.tile([128, C], f32, tag="o2")
                        if use_bo2:
                            nc.vector.tensor_add(o_sb[:], o_sb[:], bo2bc[:])
                        nc.vector.tensor_add(o2[:], o_sb[:], x_sb[:, it, :].bitcast(f32))
                        nc.sync.dma_start(
                            out_d[b, it * 128:(it + 1) * 128, :], o2[:])

    nc.compile()
    return nc


def _host_consts():
    g4 = np.zeros((128, CT * G), np.float32)
    e4 = np.zeros((G, CT * 128), np.float32)
    for ct in range(CT):
        for p in range(128):
            g = ct * 8 + p // 16
            g4[p, ct * G + g] = 1.0
            e4[g, ct * 128 + p] = 1.0
    return g4, e4, np.ones((128, 1), np.float32), np.eye(128, dtype=np.float32)


def kernel(**inputs):
    from concourse import bass_utils

    x = np.ascontiguousarray(np.asarray(inputs["x"], np.float32))
    gn_scale = np.asarray(inputs["gn_scale"], np.float32)
    gn_bias = np.asarray(inputs["gn_bias"], np.float32)
    Wq = np.ascontiguousarray(np.asarray(inputs["Wq"], np.float32))
    Wk = np.ascontiguousarray(np.asarray(inputs["Wk"], np.float32))
    Wv = np.ascontiguousarray(np.asarray(inputs["Wv"], np.float32))
    Wo = np.ascontiguousarray(np.asarray(inputs["Wo"], np.float32))
    bq = np.asarray(inputs["bq"], np.float32)
    bk = np.asarray(inputs["bk"], np.float32)
    bv = np.asarray(inputs["bv"], np.float32)
    bo = np.asarray(inputs["bo"], np.float32)

    B, H, W, Cc = x.shape
    assert (B, H * W, Cc) == (B_TOTAL, N, C)

    bo2 = bv @ Wo + bo
    use_bq = bool(np.any(bq))
    use_bk = bool(np.any(bk))
    use_bo2 = bool(np.any(bo2))

    key = (use_bq, use_bk, use_bo2)
    if key not in _CACHE:
        _CACHE[key] = _build(*key)
    nc = _CACHE[key]

    g4, e4, ones, ident = _host_consts()

    def cols(vec):
        return np.ascontiguousarray(vec.reshape(CT, 128).T)

    base = {
        "g4": g4, "e4": e4, "ones_in": ones, "ident_in": ident,
        "gnsc": cols(gn_scale), "gnbc": cols(gn_bias),
    }
    if not (use_bq or use_bk):
        base["wm"] = np.ascontiguousarray(
            (Wk.astype(np.float64) @ Wq.T.astype(np.float64)).astype(np.float32))
        base["wvo"] = np.ascontiguousarray(
            (Wv.astype(np.float64) @ Wo.astype(np.float64)).astype(np.float32))
    else:
        base.update({"wq": Wq, "wk": Wk, "wv": Wv, "wo": Wo})
    if use_bq:
        base["bqc"] = cols(bq)
    if use_bk:
        base["bkc"] = cols(bk)
    if use_bo2:
        base["bo2bc"] = np.ascontiguousarray(
            np.broadcast_to(bo2[None, :], (128, C)))

    x_flat = x.reshape(B_TOTAL, N, C)
    in_maps = []
    for c in range(N_CORES):
        m = dict(base)
        m["xs"] = np.ascontiguousarray(x_flat[c * B_PER:(c + 1) * B_PER])
        in_maps.append(m)

    res = bass_utils.run_bass_kernel_spmd(nc, in_maps,
                                          core_ids=list(range(N_CORES)))
    out = np.concatenate([r["out"] for r in res.results], axis=0)
    return out.reshape(B_TOTAL, H, W, C).astype(np.float32)
